# revision 2
# baseline (speedup 1.0000x reference)
"""Trainium2 Bass kernel for nn_MultiHeadMLPAttentionModel.

Model: per (b, n) point: pairwise = [radar_b(4), pt(2)] (radar constant over n).
  h1 = relu(pairwise @ enc_w1 + enc_b1)            [B,N,64]
  pf = h1 @ enc_w2 + enc_b2                        [B,N,64]
  sh = relu(einsum('bnf,hfd', pairwise, sc_w1) + sc_b1)
  logits = einsum('bnhd,hd', sh, sc_w2) + sc_b2    [B,N,4]
  w = softmax(logits, axis=n)
  ctx = einsum('bnh,bnd', w, pf)  -> out MLP -> [B]

Key algebraic restructurings used here:
  * pooling commutes with the (linear) second encoder layer since softmax
    weights sum to 1:  ctx = (sum_n w * h1) @ enc_w2 + enc_b2.  This removes
    the N-scale enc2 matmul entirely.
  * sc_b2 is constant over n, so it drops out of the softmax.
  * the radar part of pairwise is constant over n, so all layer-1 radar
    contributions fold into per-b bias vectors (computed on host: ~200 KFLOP
    of the model's 13 GFLOP).
  * softmax is computed without max-subtraction (logits are O(1) for this
    model; exp is evaluated in fp32) and normalization is deferred: the
    pooling matmul accumulates unnormalized sum_n exp(l)*h1 plus sum_n exp(l)
    (via an appended ones column), and the division happens once per b.

Sharding: pure data parallel over B: 8 cores x 16 rows each.  One SPMD Bass
program; per-core inputs differ only in data.
"""

import numpy as np

import concourse.bass as bass
import concourse.tile as tile
from concourse import bacc, mybir

B, N, HID, HEADS = 128, 8192, 64, 4
NCORES = 8
BPC = B // NCORES  # 16 batch rows per core
CHUNK = 512
NCH = N // CHUNK  # 16
NB = N // 128  # 64 point-blocks of 128

F32 = mybir.dt.float32
BF16 = mybir.dt.bfloat16
AF = mybir.ActivationFunctionType
ALU = mybir.AluOpType


def build_nc(reps=1, phases="ATPD"):
    from contextlib import ExitStack

    nc = bacc.Bacc()
    f32 = F32

    xp_d = nc.dram_tensor("xp", [BPC, 6, N], BF16, kind="ExternalInput")
    xpa_d = nc.dram_tensor("xpa", [NCH, 4, BPC * CHUNK], BF16, kind="ExternalInput")
    cb1_d = nc.dram_tensor("cb1", [128, BPC], f32, kind="ExternalInput")
    cb2_d = nc.dram_tensor("cb2", [128, BPC], f32, kind="ExternalInput")
    wp_d = nc.dram_tensor("wp", [4, 256], BF16, kind="ExternalInput")
    w2a_d = nc.dram_tensor("w2a", [128, BPC * 64], BF16, kind="ExternalInput")
    w2b_d = nc.dram_tensor("w2b", [128, BPC * 64], BF16, kind="ExternalInput")
    wenm_d = nc.dram_tensor("wenm", [6, BPC * 65], BF16, kind="ExternalInput")
    ew2b_d = nc.dram_tensor("ew2b", [65, 64], f32, kind="ExternalInput")
    ow1_d = nc.dram_tensor("ow1", [64, 256], f32, kind="ExternalInput")
    ob1_d = nc.dram_tensor("ob1", [1, 64], f32, kind="ExternalInput")
    w2o_d = nc.dram_tensor("w2o", [65, 1], f32, kind="ExternalInput")
    id64_d = nc.dram_tensor("id64", [64, 64], BF16, kind="ExternalInput")
    id64f_d = nc.dram_tensor("id64f", [64, 64], f32, kind="ExternalInput")
    on16_d = nc.dram_tensor("on16", [1, BPC], f32, kind="ExternalInput")
    out_d = nc.dram_tensor("out", [BPC], f32, kind="ExternalOutput")

    with tile.TileContext(nc) as tc, ExitStack() as ctx:
        consts = ctx.enter_context(tc.tile_pool(name="consts", bufs=1))

        def cload(dram, shape, nm, dt=f32):
            t = consts.tile(shape, dt, name=nm, tag=nm)
            nc.sync.dma_start(t[:], dram[:])
            return t

        wp_s = cload(wp_d, [4, 256], "wp_s", BF16)
        cb1_s = cload(cb1_d, [128, BPC], "cb1_s")
        cb2_s = cload(cb2_d, [128, BPC], "cb2_s")
        w2a_s = cload(w2a_d, [128, BPC * 64], "w2a_s", BF16)
        w2b_s = cload(w2b_d, [128, BPC * 64], "w2b_s", BF16)
        wenm_s = cload(wenm_d, [6, BPC * 65], "wenm_s", BF16)
        ew2b_s = cload(ew2b_d, [65, 64], "ew2b_s")
        ow1_s = cload(ow1_d, [64, 256], "ow1_s")
        ob1_s = cload(ob1_d, [1, 64], "ob1_s")
        w2o_s = cload(w2o_d, [65, 1], "w2o_s")
        id64_s = cload(id64_d, [64, 64], "id64_s", BF16)
        id64f_s = cload(id64f_d, [64, 64], "id64f_s")
        on16_s = cload(on16_d, [1, BPC], "on16_s")

        # n-major exp(logits): block t occupies cols [t*64, (t+1)*64), within a
        # block: partition p = n offset, col = 4*b + h
        enm = consts.tile([128, NB * 64], BF16, name="enm", tag="enm")
        ctxnT = consts.tile([65, 64], f32, name="ctxnT", tag="ctxnT")
        obuf = consts.tile([65, BPC], f32, name="obuf", tag="obuf")
        fct = consts.tile([64, 64], f32, name="fct", tag="fct")
        res = consts.tile([1, BPC], f32, name="res", tag="res")
        nc.vector.memset(ctxnT[64:65, :], 1.0)
        nc.vector.memset(obuf[64:65, :], 1.0)

        if "A" not in phases:
            nc.vector.memset(enm[:, 0:8], 0.0)
        for _rep in range(reps):
            _build_body(
                nc, tc, xp_d, xpa_d, out_d,
                wp_s, cb1_s, cb2_s, w2a_s, w2b_s, wenm_s, ew2b_s, ow1_s,
                ob1_s, w2o_s, id64_s, id64f_s, on16_s,
                enm, ctxnT, obuf, fct, res, phases,
            )

    if not nc.is_finalized():
        nc.finalize()
    return nc


def _build_body(
    nc, tc, xp_d, xpa_d, out_d,
    wp_s, cb1_s, cb2_s, w2a_s, w2b_s, wenm_s, ew2b_s, ow1_s,
    ob1_s, w2o_s, id64_s, id64f_s, on16_s,
    enm, ctxnT, obuf, fct, res, phases="ATPD",
):
    from contextlib import ExitStack

    f32 = F32
    if "A" in phases:
        # ---- Phase A: score-net hidden + logits (feature-major) ----------
        with ExitStack() as pctx:
            xpool = pctx.enter_context(tc.tile_pool(name="xpA", bufs=3))
            shpool = pctx.enter_context(tc.tile_pool(name="shp", bufs=4))
            epool = pctx.enter_context(tc.tile_pool(name="ep", bufs=2))
            psA = pctx.enter_context(tc.tile_pool(name="psA", bufs=2, space="PSUM"))
            psL = pctx.enter_context(tc.tile_pool(name="psL", bufs=2, space="PSUM"))
            psT = pctx.enter_context(tc.tile_pool(name="psT", bufs=2, space="PSUM"))

            xpcs = {}

            def load_xpc(c):
                t = xpool.tile([4, BPC * CHUNK], BF16, name="xpc", tag="xpc")
                nc.sync.dma_start(t[:], xpa_d[c])
                xpcs[c] = t

            DEPTH = 2  # software-pipeline depth: sh-MMs run DEPTH b's ahead
            lg_done = {}

            def expose(c):
                # exp of chunk c's logits, then transpose its 4 blocks n-major
                lg = lg_done.pop(c)
                e_c = epool.tile([64, CHUNK], BF16, name="e_c", tag="e_c")
                nc.scalar.activation(e_c[:], lg[:], AF.Exp)
                for j in range(CHUNK // 128):
                    t = c * (CHUNK // 128) + j
                    t_ps = psT.tile([128, 64], BF16, name="t_ps", tag="tp")
                    nc.tensor.transpose(
                        t_ps[:], e_c[:, j * 128 : (j + 1) * 128], id64_s[:]
                    )
                    nc.vector.tensor_copy(
                        out=enm[:, t * 64 : (t + 1) * 64], in_=t_ps[:]
                    )

            load_xpc(0)
            if NCH > 1:
                load_xpc(1)
            for c in range(NCH):
                cs = slice(c * CHUNK, (c + 1) * CHUNK)
                if c + 2 < NCH:
                    load_xpc(c + 2)
                if c > 0:
                    expose(c - 1)
                xpc = xpcs.pop(c)
                lg_ps = psL.tile([64, CHUNK], f32, name="lg_ps", tag="lg")
                pend = []

                def drain_lg(lg_ps=lg_ps):
                    b, s1, s2 = pend.pop(0)
                    nc.tensor.matmul(
                        lg_ps[:],
                        w2a_s[:, b * 64 : (b + 1) * 64],
                        s1[:],
                        start=(b == 0),
                        stop=False,
                        skip_group_check=True,
                    )
                    nc.tensor.matmul(
                        lg_ps[:],
                        w2b_s[:, b * 64 : (b + 1) * 64],
                        s2[:],
                        start=False,
                        stop=(b == BPC - 1),
                        skip_group_check=True,
                    )

                for b in range(BPC):
                    xb = xpc[:, b * CHUNK : (b + 1) * CHUNK]
                    sh1_ps = psA.tile([128, CHUNK], f32, name="sh1_ps", tag="sh1")
                    nc.tensor.matmul(
                        sh1_ps[:], wp_s[:, 0:128], xb[:, :], start=True, stop=True
                    )
                    sh2_ps = psA.tile([128, CHUNK], f32, name="sh2_ps", tag="sh2")
                    nc.tensor.matmul(
                        sh2_ps[:], wp_s[:, 128:256], xb[:, :], start=True, stop=True
                    )
                    sh1_sb = shpool.tile([128, CHUNK], BF16, name="sh1_sb", tag="sh1s")
                    sh2_sb = shpool.tile([128, CHUNK], BF16, name="sh2_sb", tag="sh2s")
                    if b % 2 == 0:
                        nc.scalar.activation(
                            sh1_sb[:], sh1_ps[:], AF.Relu, bias=cb1_s[:, b : b + 1]
                        )
                        nc.vector.tensor_scalar(
                            sh2_sb[:], sh2_ps[:], cb2_s[:, b : b + 1], 0.0,
                            ALU.add, ALU.max,
                        )
                    else:
                        nc.vector.tensor_scalar(
                            sh1_sb[:], sh1_ps[:], cb1_s[:, b : b + 1], 0.0,
                            ALU.add, ALU.max,
                        )
                        nc.scalar.activation(
                            sh2_sb[:], sh2_ps[:], AF.Relu, bias=cb2_s[:, b : b + 1]
                        )
                    pend.append((b, sh1_sb, sh2_sb))
                    if len(pend) > DEPTH:
                        drain_lg()
                while pend:
                    drain_lg()
                lg_done[c] = lg_ps
            expose(NCH - 1)

    if "P" in phases:
        # ---- Phase C2: n-major encoder hidden + weighted pooling ---------
        with ExitStack() as pctx:
            xbpool = pctx.enter_context(tc.tile_pool(name="xpC", bufs=2))
            h1pool = pctx.enter_context(tc.tile_pool(name="h1p", bufs=3))
            smpool = pctx.enter_context(tc.tile_pool(name="smp", bufs=2))
            psH = pctx.enter_context(tc.tile_pool(name="psH", bufs=3, space="PSUM"))
            psC = pctx.enter_context(tc.tile_pool(name="psC", bufs=2, space="PSUM"))
            psU = pctx.enter_context(tc.tile_pool(name="psU", bufs=2, space="PSUM"))
            TB = 4  # blocks per psum batch
            xpbs = {}

            def load_xpb(b):
                t = xbpool.tile([6, N], BF16, name="xpb", tag="xpb")
                nc.sync.dma_start(t[:], xp_d[b])
                xpbs[b] = t

            load_xpb(0)
            for b in range(BPC):
                if b + 1 < BPC:
                    load_xpb(b + 1)
                xpb = xpbs.pop(b)
                c1_ps = psC.tile([4, 65], f32, name="c1_ps", tag="c1")
                hpend = []

                def drain_pool(c1_ps=c1_ps, b=b):
                    tg, h1_sb = hpend.pop(0)
                    for j in range(TB):
                        t = tg * TB + j
                        nc.tensor.matmul(
                            c1_ps[:],
                            enm[:, t * 64 + 4 * b : t * 64 + 4 * b + 4],
                            h1_sb[:, j * 65 : (j + 1) * 65],
                            start=(t == 0),
                            stop=(t == NB - 1),
                            skip_group_check=True,
                        )

                for tg in range(NB // TB):
                    h1_ps = psH.tile([128, TB * 65], f32, name="h1_ps", tag="h1")
                    for j in range(TB):
                        t = tg * TB + j
                        nc.tensor.matmul(
                            h1_ps[:, j * 65 : (j + 1) * 65],
                            xpb[:, t * 128 : (t + 1) * 128],
                            wenm_s[:, b * 65 : (b + 1) * 65],
                            start=True,
                            stop=True,
                            skip_group_check=True,
                        )
                    h1_sb = h1pool.tile([128, TB * 65], BF16, name="h1_sb", tag="h1s")
                    nc.vector.tensor_scalar(
                        h1_sb[:], h1_ps[:], 0.0, None, ALU.max
                    )
                    hpend.append((tg, h1_sb))
                    if len(hpend) > 1:
                        drain_pool()
                while hpend:
                    drain_pool()
                rz = smpool.tile([4, 1], f32, name="rz", tag="rz")
                nc.vector.reciprocal(rz[:], c1_ps[:, 64:65])
                ctxn = smpool.tile([4, 64], f32, name="ctxn", tag="ctxn")
                nc.vector.tensor_scalar_mul(ctxn[:], c1_ps[:, 0:64], rz[:])
                tp_ps = psU.tile([64, 4], f32, name="tp_ps", tag="tp2")
                nc.tensor.transpose(tp_ps[:], ctxn[:], id64f_s[0:4, 0:4])
                nc.vector.tensor_copy(
                    out=ctxnT[0:64, b * 4 : (b + 1) * 4], in_=tp_ps[:]
                )

    if "D" in phases:
        # ---- Phase D: pooled-context encoder layer 2 + output MLP --------
        with ExitStack() as pctx:
            psD = pctx.enter_context(tc.tile_pool(name="psD", bufs=1, space="PSUM"))
            fct_ps = psD.tile([64, 64], f32, name="fct_ps", tag="fctp")
            nc.tensor.matmul(fct_ps[:], ew2b_s[:], ctxnT[:], start=True, stop=True)
            nc.vector.tensor_copy(out=fct[:], in_=fct_ps[:])
            fct_bh = fct.rearrange("d (b h) -> d b h", h=HEADS)
            o1_ps = psD.tile([64, BPC], f32, name="o1_ps", tag="o1p")
            for h in range(HEADS):
                nc.tensor.matmul(
                    o1_ps[:],
                    ow1_s[:, h * 64 : (h + 1) * 64],
                    fct_bh[:, :, h],
                    start=(h == 0),
                    stop=False,
                    skip_group_check=True,
                )
            nc.tensor.matmul(
                o1_ps[:], ob1_s[:], on16_s[:], start=False, stop=True,
                skip_group_check=True,
            )
            nc.scalar.activation(obuf[0:64, :], o1_ps[:], AF.Relu)
            fin_ps = psD.tile([1, BPC], f32, name="fin_ps", tag="finp")
            nc.tensor.matmul(fin_ps[:], w2o_s[:], obuf[:], start=True, stop=True)
            nc.vector.tensor_copy(out=res[:], in_=fin_ps[:])
            nc.sync.dma_start(out_d.rearrange("(a n) -> a n", a=1), res[:])


def make_in_maps(inputs):
    """Host-side marshalling: slice B across cores and pack weights into the
    layouts the device program expects.

    bf16 note: the big streamed matmuls run in bf16.  To avoid systematic
    model-weight rounding, layer-1 weights are split hi/lo across extra
    contraction rows (w = hi + lo with both bf16); per-point input rounding
    is stochastic and averages out in the softmax pooling."""
    import ml_dtypes

    bf = ml_dtypes.bfloat16
    f = np.float32

    def split(a):
        hi = a.astype(bf)
        lo = (a - hi.astype(f)).astype(bf)
        return hi, lo
    radar = np.concatenate(
        [np.asarray(inputs["radar_xy"], f), np.asarray(inputs["radar_dir"], f)], axis=1
    )  # [B, 4]
    pts = np.asarray(inputs["pts"], f)
    enc_w1 = np.asarray(inputs["enc_w1"], f)
    enc_b1 = np.asarray(inputs["enc_b1"], f)
    enc_w2 = np.asarray(inputs["enc_w2"], f)
    enc_b2 = np.asarray(inputs["enc_b2"], f)
    sc_w1 = np.asarray(inputs["sc_w1"], f)
    sc_b1 = np.asarray(inputs["sc_b1"], f)
    sc_w2 = np.asarray(inputs["sc_w2"], f)
    out_w1 = np.asarray(inputs["out_w1"], f)
    out_b1 = np.asarray(inputs["out_b1"], f)
    out_w2 = np.asarray(inputs["out_w2"], f)
    out_b2 = np.asarray(inputs["out_b2"], f)

    # per-b layer-1 bias vectors (radar is constant over n)
    cb_sc = np.einsum("br,hrd->bhd", radar, sc_w1[:, :4, :]) + sc_b1  # [B, 4, 64]
    cb_enc = radar @ enc_w1[:4] + enc_b1  # [B, 64]

    # xp rows: [xh, yh, xh, yh, 1, 1] (bf16); rows 0-3 feed the weight-split
    # layer-1 matmuls, rows 4-5 carry the (split) bias contraction.
    xp = np.empty((B, 6, N), bf)
    xh = pts[:, :, 0].astype(bf)
    yh = pts[:, :, 1].astype(bf)
    xp[:, 0] = xh
    xp[:, 1] = yh
    xp[:, 2] = xh
    xp[:, 3] = yh
    xp[:, 4] = 1.0
    xp[:, 5] = 1.0

    # wp rows: [wxh, wyh, wxl, wyl] against xp rows [xh, yh, xh, yh]
    wp = np.empty((4, 256), bf)
    for h in range(HEADS):
        wxh, wxl = split(sc_w1[h, 4, :])
        wyh, wyl = split(sc_w1[h, 5, :])
        wp[0, h * 64 : (h + 1) * 64] = wxh
        wp[1, h * 64 : (h + 1) * 64] = wyh
        wp[2, h * 64 : (h + 1) * 64] = wxl
        wp[3, h * 64 : (h + 1) * 64] = wyl
    # heads 0,1 feed sh1 (wp cols 0:128), heads 2,3 feed sh2 (cols 128:256)

    w2a = np.zeros((128, BPC * 64), bf)
    w2b = np.zeros((128, BPC * 64), bf)
    for bl in range(BPC):
        w2a[0:64, bl * 64 + 4 * bl + 0] = sc_w2[0]
        w2a[64:128, bl * 64 + 4 * bl + 1] = sc_w2[1]
        w2b[0:64, bl * 64 + 4 * bl + 2] = sc_w2[2]
        w2b[64:128, bl * 64 + 4 * bl + 3] = sc_w2[3]

    ew2b = np.concatenate([enc_w2, enc_b2[None, :]], axis=0)  # [65, 64]
    ow1 = np.empty((64, 256), f)
    for h in range(HEADS):
        ow1[:, h * 64 : (h + 1) * 64] = out_w1[h * 64 : (h + 1) * 64, :]
    ob1 = np.ascontiguousarray(out_b1[None, :])
    w2o = np.concatenate([out_w2, out_b2[None, :]], axis=0)  # [65, 1]
    id64 = np.eye(64, dtype=bf)
    id64f = np.eye(64, dtype=f)
    on16 = np.ones((1, BPC), f)

    in_maps = []
    for c in range(NCORES):
        sl = slice(c * BPC, (c + 1) * BPC)
        cb1 = np.ascontiguousarray(cb_sc[sl, 0:2].reshape(BPC, 128).T)
        cb2 = np.ascontiguousarray(cb_sc[sl, 2:4].reshape(BPC, 128).T)
        # wenm rows [wxh, wyh, wxl, wyl, bh, bl] vs xp rows [xh, yh, xh, yh, 1, 1]
        wenm = np.zeros((6, BPC * 65), bf)
        exh, exl = split(enc_w1[4])
        eyh, eyl = split(enc_w1[5])
        for bl in range(BPC):
            s = slice(bl * 65, bl * 65 + 64)
            wenm[0, s] = exh
            wenm[1, s] = eyh
            wenm[2, s] = exl
            wenm[3, s] = eyl
            bh, blo = split(cb_enc[c * BPC + bl])
            wenm[4, s] = bh
            wenm[5, s] = blo
            wenm[4, bl * 65 + 64] = 1.0
        xpc_core = np.ascontiguousarray(xp[sl])
        xpa = np.ascontiguousarray(
            xpc_core[:, 0:4]
            .reshape(BPC, 4, NCH, CHUNK)
            .transpose(2, 1, 0, 3)
            .reshape(NCH, 4, BPC * CHUNK)
        )
        in_maps.append(
            dict(
                xp=xpc_core,
                xpa=xpa,
                cb1=cb1,
                cb2=cb2,
                wp=wp,
                w2a=w2a,
                w2b=w2b,
                wenm=wenm,
                ew2b=ew2b,
                ow1=ow1,
                ob1=ob1,
                w2o=w2o,
                id64=id64,
                id64f=id64f,
                on16=on16,
            )
        )
    return in_maps


_CACHE = {}


def _get_runner():
    """Build the Bass program once and a cached jitted PJRT executable over
    the 8 cores (shard_map along axis 0 of every input)."""
    if "runner" in _CACHE:
        return _CACHE["runner"]

    import jax
    from jax.sharding import Mesh, NamedSharding, PartitionSpec

    from concourse.bass2jax import (
        _bass_exec_p,
        install_neuronx_cc_hook,
        partition_id_tensor,
        shard_map,
    )

    nc = build_nc()
    _CACHE["nc"] = nc
    install_neuronx_cc_hook()
    partition_name = nc.partition_id_tensor.name if nc.partition_id_tensor else None
    in_names, out_names, out_avals = [], [], []
    for alloc in nc.m.functions[0].allocations:
        if not isinstance(alloc, mybir.MemoryLocationSet):
            continue
        name = alloc.memorylocations[0].name
        if alloc.kind == "ExternalInput":
            if name != partition_name:
                in_names.append(name)
        elif alloc.kind == "ExternalOutput":
            out_names.append(name)
            out_avals.append(
                jax.core.ShapedArray(tuple(alloc.tensor_shape), mybir.dt.np(alloc.dtype))
            )
    all_in_names = tuple(in_names + out_names)
    if partition_name is not None:
        all_in_names = all_in_names + (partition_name,)

    def _body(*args):
        operands = list(args)
        if partition_name is not None:
            operands.append(partition_id_tensor())
        return tuple(
            _bass_exec_p.bind(
                *operands,
                out_avals=tuple(out_avals),
                in_names=all_in_names,
                out_names=tuple(out_names),
                lowering_input_output_aliases=(),
                sim_require_finite=True,
                sim_require_nnan=True,
                nc=nc,
            )
        )

    devices = jax.devices()[:NCORES]
    mesh = Mesh(np.asarray(devices), ("core",))
    nin = len(in_names) + len(out_names)
    fn = jax.jit(
        shard_map(
            _body,
            mesh=mesh,
            in_specs=(PartitionSpec("core"),) * nin,
            out_specs=(PartitionSpec("core"),) * len(out_names),
            check_rep=False,
        ),
        keep_unused=True,
    )
    sharding = NamedSharding(mesh, PartitionSpec("core"))
    runner = (fn, sharding, in_names, out_avals)
    _CACHE["runner"] = runner
    return runner


def kernel(**inputs):
    import jax

    in_maps = make_in_maps(inputs)
    fn, sharding, in_names, out_avals = _get_runner()
    concat_in = [
        np.concatenate([np.asarray(in_maps[c][name]) for c in range(NCORES)], axis=0)
        for name in in_names
    ]
    concat_zeros = [
        np.zeros((NCORES * a.shape[0], *a.shape[1:]), a.dtype) for a in out_avals
    ]
    args = [jax.device_put(a, sharding) for a in (*concat_in, *concat_zeros)]
    (out,) = fn(*args)
    return np.asarray(out).reshape(B).astype(np.float32)



# revision 6
# speedup vs baseline: 2.8729x; 2.8729x over previous
"""Trainium2 Bass kernel for nn_MultiHeadMLPAttentionModel (v2).

Model (per b, per point n): pairwise = [radar_b(4), pt(2)]; radar constant
over n folds into per-(b,head) biases computed on host.

  sh    = relu(Wsc pt + cb)      score-net hidden, 4 heads x 64   [N,256]
  logit = w2 . sh                per head                          [N,4]
  h1    = relu(We pt + cbe)      encoder hidden                    [N,64]
  w     = softmax(logit, n);  ctx = sum_n w * h1  (then enc2+MLP on pooled)

v2 design (calibrated on HW microbenchmarks):
  * inputs + layer-1 weights in fp8e4 with hi/lo weight splitting; host-sim
    rel err 2.8e-3 vs fp32 reference (tolerance 2e-2).
  * score-hidden production: [K=8,M=128,N=512] matmuls row-packed 4x via
    tile_position=(32j,0) -> ~107ns each (4 concurrent PE row-groups).
    Data is DMA-replicated at partition offsets 0/32/64/96 to feed them.
  * logits: n-major drains, lhsT = sh-block [128,128] (FWL weight loads),
    rhs = w2 columns [128,2]; output lands points-major so softmax-exp
    output feeds pooling with no transposes.
  * pooling: lhsT = exp-weights [128pts,4heads], rhs = h1n [128,65];
    4 batch rows col-packed per PSUM bank via tile_position=(0,32r);
    accumulated across all 64 point-blocks in PSUM.
  * relu-copies (the throughput floor) split across DVE and ACT every unit.
  * 2 passes x 8 batch rows to fit PSUM (3 sh + 1 lg + 2 h1 + 2 ctx = 8 banks).

Sharding: pure data parallel over B: 8 cores x 16 rows.
"""

import numpy as np

import concourse.bass as bass
import concourse.tile as tile
from concourse import bacc, mybir

B, N, HID, HEADS = 128, 8192, 64, 4
NCORES = 8
BPC = B // NCORES      # 16 batch rows per core
CHUNK = 512
NCH = N // CHUNK       # 16
PASSES = 2
BPP = BPC // PASSES    # 8 batch rows per pass
import os
ROWPACK = os.environ.get("KV2_ROWPACK", "1") == "1"
ROWPACK_H1 = os.environ.get("KV2_ROWPACK_H1", "1") == "1"
COLPACK = os.environ.get("KV2_COLPACK", "1") == "1"

F32 = mybir.dt.float32
BF16 = mybir.dt.bfloat16
FP8 = mybir.dt.float8e4
AF = mybir.ActivationFunctionType
ALU = mybir.AluOpType


def build_nc():
    from contextlib import ExitStack

    nc = bacc.Bacc()
    f32 = F32

    # point data, n-major, 4 partition-group replicas of 8 rows
    xpn_d = nc.dram_tensor("xpn", [PASSES, NCH, 4, 8, BPP * CHUNK], FP8,
                           kind="ExternalInput")
    wsc_d = nc.dram_tensor("wsc", [4, 8, BPC * 2 * 128], FP8, kind="ExternalInput")
    wenc_d = nc.dram_tensor("wenc", [4, 8, BPC * 65], FP8, kind="ExternalInput")
    w2n_d = nc.dram_tensor("w2n", [128, 4], BF16, kind="ExternalInput")
    ew2b_d = nc.dram_tensor("ew2b", [65, 64], f32, kind="ExternalInput")
    ow1_d = nc.dram_tensor("ow1", [64, 256], f32, kind="ExternalInput")
    ob1_d = nc.dram_tensor("ob1", [1, 64], f32, kind="ExternalInput")
    w2o_d = nc.dram_tensor("w2o", [65, 1], f32, kind="ExternalInput")
    id64f_d = nc.dram_tensor("id64f", [64, 64], f32, kind="ExternalInput")
    on16_d = nc.dram_tensor("on16", [1, BPC], f32, kind="ExternalInput")
    out_d = nc.dram_tensor("out", [BPC], f32, kind="ExternalOutput")

    with tile.TileContext(nc) as tc, ExitStack() as ctx:
        consts = ctx.enter_context(tc.tile_pool(name="consts", bufs=1))

        wsc_s = consts.tile([128, BPC * 2 * 128], FP8, name="wsc_s", tag="wsc_s")
        wenc_s = consts.tile([128, BPC * 65], FP8, name="wenc_s", tag="wenc_s")
        for j in range(4):
            nc.sync.dma_start(wsc_s[32 * j : 32 * j + 8, :], wsc_d[j])
            nc.sync.dma_start(wenc_s[32 * j : 32 * j + 8, :], wenc_d[j])

        def cload(dram, shape, nm, dt=f32):
            t = consts.tile(shape, dt, name=nm, tag=nm)
            nc.sync.dma_start(t[:], dram[:])
            return t

        w2n_s = cload(w2n_d, [128, 4], "w2n_s", BF16)
        ew2b_s = cload(ew2b_d, [65, 64], "ew2b_s")
        ow1_s = cload(ow1_d, [64, 256], "ow1_s")
        ob1_s = cload(ob1_d, [1, 64], "ob1_s")
        w2o_s = cload(w2o_d, [65, 1], "w2o_s")
        id64f_s = cload(id64f_d, [64, 64], "id64f_s")
        on16_s = cload(on16_d, [1, BPC], "on16_s")

        ctxnT = consts.tile([65, 64], f32, name="ctxnT", tag="ctxnT")
        obuf = consts.tile([65, BPC], f32, name="obuf", tag="obuf")
        fct = consts.tile([64, 64], f32, name="fct", tag="fct")
        res = consts.tile([1, BPC], f32, name="res", tag="res")
        nc.vector.memset(ctxnT[64:65, :], 1.0)
        nc.vector.memset(obuf[64:65, :], 1.0)

        for p in range(PASSES):
            with ExitStack() as passctx:
                ctxps = passctx.enter_context(
                    tc.tile_pool(name=f"ctxps{p}", bufs=1, space="PSUM"))
                ctx_t = [ctxps.tile([128, 65], f32, name=f"ctx{q}", tag=f"ctx{q}")
                         for q in range(2)]

                with ExitStack() as cctx:
                    xpool = cctx.enter_context(tc.tile_pool(name="xp", bufs=3))
                    shsb = cctx.enter_context(tc.tile_pool(name="shsb", bufs=3))
                    h1sb = cctx.enter_context(tc.tile_pool(name="h1sb", bufs=2))
                    esb = cctx.enter_context(tc.tile_pool(name="esb", bufs=2))
                    shps = cctx.enter_context(
                        tc.tile_pool(name="shps", bufs=3, space="PSUM"))
                    lgps = cctx.enter_context(
                        tc.tile_pool(name="lgps", bufs=1, space="PSUM"))
                    h1ps = cctx.enter_context(
                        tc.tile_pool(name="h1ps", bufs=2, space="PSUM"))

                    xqs = {}

                    def load_x(c, p=p):
                        t = xpool.tile([128, BPP * CHUNK], FP8, name="xq", tag="xq")
                        for j in range(4):
                            nc.sync.dma_start(t[32 * j : 32 * j + 8, :],
                                              xpn_d[p, c, j])
                        xqs[c] = t

                    load_x(0)
                    if NCH > 1:
                        load_x(1)
                    e_prev = None
                    h1_prev = None

                    for c in range(NCH + 1):
                        if c < NCH:
                            if c + 2 < NCH:
                                load_x(c + 2)
                            xq = xqs.pop(c)
                            lg = lgps.tile([128, BPP * 16], f32, name="lg", tag="lg")
                            h1_cur = []
                            for u in range(BPP):
                                bb = p * BPP + u
                                us = slice(u * CHUNK, (u + 1) * CHUNK)
                                # --- score hidden production (row-packed) ---
                                sh_a = shps.tile([128, 512], f32, name="sha", tag="sh")
                                sh_b = shps.tile([128, 512], f32, name="shb", tag="sh")
                                for hp, sh_ps in ((0, sh_a), (1, sh_b)):
                                    j = (2 * u + hp) % 4 if ROWPACK else 0
                                    cw = (bb * 2 + hp) * 128
                                    nc.tensor.matmul(
                                        sh_ps[:],
                                        wsc_s[32 * j : 32 * j + 8, cw : cw + 128],
                                        xq[32 * j : 32 * j + 8, us],
                                        start=True, stop=True,
                                        tile_position=(32 * j, 0) if ROWPACK else None,
                                        skip_group_check=True,
                                    )
                                shs = shsb.tile([128, 1024], BF16, name="shs",
                                                tag="shs")
                                nc.vector.tensor_scalar(
                                    shs[:, 0:512], sh_a[:], 0.0, None, ALU.max)
                                nc.scalar.activation(
                                    shs[:, 512:1024], sh_b[:], AF.Relu)
                                # --- logits: n-major drains ---
                                for t in range(4):
                                    for hp in range(2):
                                        lc = u * 16 + t * 4 + hp * 2
                                        nc.tensor.matmul(
                                            lg[:, lc : lc + 2],
                                            shs[:, hp * 512 + t * 128
                                                : hp * 512 + t * 128 + 128],
                                            w2n_s[:, hp * 2 : hp * 2 + 2],
                                            start=True, stop=True,
                                            skip_group_check=True,
                                        )
                                # --- encoder hidden production (row-packed) ---
                                h1p = h1ps.tile([128, 260], f32, name="h1p",
                                                tag="h1p")
                                for t in range(4):
                                    jt = t if ROWPACK_H1 else 0
                                    nc.tensor.matmul(
                                        h1p[:, 65 * t : 65 * t + 65],
                                        xq[32 * jt : 32 * jt + 8,
                                           u * CHUNK + t * 128
                                           : u * CHUNK + t * 128 + 128],
                                        wenc_s[32 * jt : 32 * jt + 8,
                                               bb * 65 : bb * 65 + 65],
                                        start=True, stop=True,
                                        tile_position=(32 * jt, 0) if ROWPACK_H1 else None,
                                        skip_group_check=True,
                                    )
                                h1s = h1sb.tile([128, 260], BF16, name="h1s",
                                                tag=f"h1_{u}")
                                if u % 2 == 0:
                                    nc.vector.tensor_scalar(
                                        h1s[:], h1p[:], 0.0, None, ALU.max)
                                else:
                                    nc.scalar.activation(h1s[:], h1p[:], AF.Relu)
                                h1_cur.append(h1s)
                            e = esb.tile([128, BPP * 16], BF16, name="e", tag="e")
                            nc.scalar.activation(e[:], lg[:], AF.Exp)
                        if c > 0:
                            cp = c - 1
                            for u in range(BPP):
                                q, r = u // 4, u % 4
                                for t in range(4):
                                    ec = u * 16 + t * 4
                                    nc.tensor.matmul(
                                        ctx_t[q][32 * r : 32 * r + 4, :],
                                        e_prev[:, ec : ec + 4],
                                        h1_prev[u][:, 65 * t : 65 * t + 65],
                                        start=(cp == 0 and t == 0),
                                        stop=(cp == NCH - 1 and t == 3),
                                        tile_position=(0, 32 * r) if COLPACK else None,
                                        skip_group_check=True,
                                    )
                        if c < NCH:
                            e_prev = e
                            h1_prev = h1_cur

                # --- normalize + transpose ctx for this pass ---
                with ExitStack() as nctx:
                    smpool = nctx.enter_context(tc.tile_pool(name="smp", bufs=2))
                    psU = nctx.enter_context(
                        tc.tile_pool(name="psU", bufs=2, space="PSUM"))
                    for u in range(BPP):
                        b = p * BPP + u
                        q, r = u // 4, u % 4
                        rows = slice(32 * r, 32 * r + 4)
                        rz = smpool.tile([4, 1], f32, name="rz", tag="rz")
                        nc.vector.reciprocal(rz[:], ctx_t[q][rows, 64:65])
                        ctxn = smpool.tile([4, 64], f32, name="ctxn", tag="ctxn")
                        nc.vector.tensor_scalar_mul(
                            ctxn[:], ctx_t[q][rows, 0:64], rz[:])
                        tp_ps = psU.tile([64, 4], f32, name="tp_ps", tag="tp2")
                        nc.tensor.transpose(tp_ps[:], ctxn[:], id64f_s[0:4, 0:4])
                        nc.vector.tensor_copy(
                            out=ctxnT[0:64, b * 4 : (b + 1) * 4], in_=tp_ps[:])

        # ---- Phase D: pooled-context encoder layer 2 + output MLP ----
        with ExitStack() as pctx:
            psD = pctx.enter_context(tc.tile_pool(name="psD", bufs=1, space="PSUM"))
            fct_ps = psD.tile([64, 64], f32, name="fct_ps", tag="fctp")
            nc.tensor.matmul(fct_ps[:], ew2b_s[:], ctxnT[:], start=True, stop=True)
            nc.vector.tensor_copy(out=fct[:], in_=fct_ps[:])
            fct_bh = fct.rearrange("d (b h) -> d b h", h=HEADS)
            o1_ps = psD.tile([64, BPC], f32, name="o1_ps", tag="o1p")
            for h in range(HEADS):
                nc.tensor.matmul(
                    o1_ps[:],
                    ow1_s[:, h * 64 : (h + 1) * 64],
                    fct_bh[:, :, h],
                    start=(h == 0),
                    stop=False,
                    skip_group_check=True,
                )
            nc.tensor.matmul(
                o1_ps[:], ob1_s[:], on16_s[:], start=False, stop=True,
                skip_group_check=True,
            )
            nc.scalar.activation(obuf[0:64, :], o1_ps[:], AF.Relu)
            fin_ps = psD.tile([1, BPC], f32, name="fin_ps", tag="finp")
            nc.tensor.matmul(fin_ps[:], w2o_s[:], obuf[:], start=True, stop=True)
            nc.vector.tensor_copy(out=res[:], in_=fin_ps[:])
            nc.sync.dma_start(out_d.rearrange("(a n) -> a n", a=1), res[:])

    if not nc.is_finalized():
        nc.finalize()
    return nc


def make_in_maps(inputs):
    """Host-side marshalling: fp8 data/weights with hi/lo weight rows.

    Row scheme (8 rows, paired data x weights):
      data:    [xh, yh, xl, yl, xh, yh, 1, 1]
      weights: [wxh, wyh, wxh, wyh, wxl, wyl, bh, bl]
    so the product accumulates wxh*xh + wyh*yh + wxh*xl + wyh*yl + wxl*xh
    + wyl*yh + bh + bl  ~= wx*x + wy*y + bias at ~7-bit mantissa."""
    import ml_dtypes

    f8 = ml_dtypes.float8_e4m3
    f = np.float32

    def split8(a):
        hi = a.astype(f8)
        lo = (a - hi.astype(f)).astype(f8)
        return hi.astype(f), lo.astype(f)

    radar = np.concatenate(
        [np.asarray(inputs["radar_xy"], f), np.asarray(inputs["radar_dir"], f)],
        axis=1)
    pts = np.asarray(inputs["pts"], f)
    enc_w1 = np.asarray(inputs["enc_w1"], f)
    enc_b1 = np.asarray(inputs["enc_b1"], f)
    enc_w2 = np.asarray(inputs["enc_w2"], f)
    enc_b2 = np.asarray(inputs["enc_b2"], f)
    sc_w1 = np.asarray(inputs["sc_w1"], f)
    sc_b1 = np.asarray(inputs["sc_b1"], f)
    sc_w2 = np.asarray(inputs["sc_w2"], f)
    out_w1 = np.asarray(inputs["out_w1"], f)
    out_b1 = np.asarray(inputs["out_b1"], f)
    out_w2 = np.asarray(inputs["out_w2"], f)
    out_b2 = np.asarray(inputs["out_b2"], f)

    cb_sc = np.einsum("br,hrd->bhd", radar, sc_w1[:, :4, :]) + sc_b1  # [B,4,64]
    cb_enc = radar @ enc_w1[:4] + enc_b1                              # [B,64]

    # fp8 hi/lo of point coords, data rows [xh, yh, xl, yl, xh, yh, 1, 1]
    xh, xl = split8(pts[:, :, 0])
    yh, yl = split8(pts[:, :, 1])
    xrows = np.stack([xh, yh, xl, yl, xh, yh,
                      np.ones_like(xh), np.ones_like(xh)], axis=1)  # [B,8,N]

    def wrows(wx, wy, bias):
        # -> [8, 64] f32: [wxh, wyh, wxh, wyh, wxl, wyl, bh, bl]
        wxh, wxl = split8(wx)
        wyh, wyl = split8(wy)
        bh, bl = split8(bias)
        return np.stack([wxh, wyh, wxh, wyh, wxl, wyl, bh, bl], axis=0)

    ew2b = np.concatenate([enc_w2, enc_b2[None, :]], axis=0)
    ow1 = np.empty((64, 256), f)
    for h in range(HEADS):
        ow1[:, h * 64 : (h + 1) * 64] = out_w1[h * 64 : (h + 1) * 64, :]
    ob1 = np.ascontiguousarray(out_b1[None, :])
    w2o = np.concatenate([out_w2, out_b2[None, :]], axis=0)
    id64f = np.eye(64, dtype=f)
    on16 = np.ones((1, BPC), f)

    bf = ml_dtypes.bfloat16
    w2n = np.zeros((128, 4), bf)
    w2n[0:64, 0] = sc_w2[0]
    w2n[64:128, 1] = sc_w2[1]
    w2n[0:64, 2] = sc_w2[2]
    w2n[64:128, 3] = sc_w2[3]

    in_maps = []
    for core in range(NCORES):
        sl = slice(core * BPC, (core + 1) * BPC)
        # xpn [PASSES, NCH, 4, 8, BPP*CHUNK]
        xr = xrows[sl]  # [16, 8, N]
        xpn = np.empty((PASSES, NCH, 4, 8, BPP * CHUNK), f8)
        for p in range(PASSES):
            for c in range(NCH):
                seg = xr[p * BPP : (p + 1) * BPP, :,
                         c * CHUNK : (c + 1) * CHUNK]      # [8b, 8rows, 512]
                flat = seg.transpose(1, 0, 2).reshape(8, BPP * CHUNK)
                xpn[p, c] = flat[None, :, :].astype(f8)    # replicate 4x
        # wsc [4, 8, BPC*2*128]
        wsc = np.zeros((8, BPC * 2 * 128), f)
        wenc = np.zeros((8, BPC * 65), f)
        for bl in range(BPC):
            b = core * BPC + bl
            for hp in range(2):
                w8 = np.concatenate(
                    [wrows(sc_w1[2 * hp + i, 4], sc_w1[2 * hp + i, 5],
                           cb_sc[b, 2 * hp + i]) for i in range(2)],
                    axis=1)  # [8, 128]
                wsc[:, (bl * 2 + hp) * 128 : (bl * 2 + hp + 1) * 128] = w8
            wenc[:, bl * 65 : bl * 65 + 64] = wrows(enc_w1[4], enc_w1[5],
                                                    cb_enc[b])
            wenc[6, bl * 65 + 64] = 1.0  # normalizer column
        wsc8 = np.broadcast_to(wsc.astype(f8), (4, 8, BPC * 2 * 128)).copy()
        wenc8 = np.broadcast_to(wenc.astype(f8), (4, 8, BPC * 65)).copy()
        in_maps.append(
            dict(xpn=xpn, wsc=wsc8, wenc=wenc8, w2n=w2n, ew2b=ew2b, ow1=ow1,
                 ob1=ob1, w2o=w2o, id64f=id64f, on16=on16))
    return in_maps


_CACHE = {}


def _get_runner():
    """Build the Bass program once and a cached jitted PJRT executable over
    the 8 cores (shard_map along axis 0 of every input)."""
    if "runner" in _CACHE:
        return _CACHE["runner"]

    import jax
    from jax.sharding import Mesh, NamedSharding, PartitionSpec

    from concourse.bass2jax import (
        _bass_exec_p,
        install_neuronx_cc_hook,
        partition_id_tensor,
        shard_map,
    )

    nc = build_nc()
    _CACHE["nc"] = nc
    install_neuronx_cc_hook()
    partition_name = nc.partition_id_tensor.name if nc.partition_id_tensor else None
    in_names, out_names, out_avals = [], [], []
    for alloc in nc.m.functions[0].allocations:
        if not isinstance(alloc, mybir.MemoryLocationSet):
            continue
        name = alloc.memorylocations[0].name
        if alloc.kind == "ExternalInput":
            if name != partition_name:
                in_names.append(name)
        elif alloc.kind == "ExternalOutput":
            out_names.append(name)
            out_avals.append(
                jax.core.ShapedArray(tuple(alloc.tensor_shape), mybir.dt.np(alloc.dtype))
            )
    all_in_names = tuple(in_names + out_names)
    if partition_name is not None:
        all_in_names = all_in_names + (partition_name,)

    def _body(*args):
        operands = list(args)
        if partition_name is not None:
            operands.append(partition_id_tensor())
        return tuple(
            _bass_exec_p.bind(
                *operands,
                out_avals=tuple(out_avals),
                in_names=all_in_names,
                out_names=tuple(out_names),
                lowering_input_output_aliases=(),
                sim_require_finite=True,
                sim_require_nnan=True,
                nc=nc,
            )
        )

    devices = jax.devices()[:NCORES]
    mesh = Mesh(np.asarray(devices), ("core",))
    nin = len(in_names) + len(out_names)
    fn = jax.jit(
        shard_map(
            _body,
            mesh=mesh,
            in_specs=(PartitionSpec("core"),) * nin,
            out_specs=(PartitionSpec("core"),) * len(out_names),
            check_rep=False,
        ),
        keep_unused=True,
    )
    sharding = NamedSharding(mesh, PartitionSpec("core"))
    runner = (fn, sharding, in_names, out_avals)
    _CACHE["runner"] = runner
    return runner


def kernel(**inputs):
    import jax

    in_maps = make_in_maps(inputs)
    fn, sharding, in_names, out_avals = _get_runner()
    concat_in = [
        np.concatenate([np.asarray(in_maps[c][name]) for c in range(NCORES)], axis=0)
        for name in in_names
    ]
    concat_zeros = [
        np.zeros((NCORES * a.shape[0], *a.shape[1:]), a.dtype) for a in out_avals
    ]
    args = [jax.device_put(a, sharding) for a in (*concat_in, *concat_zeros)]
    (out,) = fn(*args)
    return np.asarray(out).reshape(B).astype(np.float32)
